# revision 1
# baseline (speedup 1.0000x reference)
"""DANetHead (position attention + channel attention + conv/BN/ReLU) on 8
Trainium2 NeuronCores via Bass/Tile.

Sharding: data-parallel over batch (4) x image-row-halves (2) = 8 cores.
Each core computes a 34-row window (32 own rows + 1 halo row on each side)
of one batch item.  The window is made position-uniform across cores by
rolling the pixel axis host-side, so a single SPMD program serves all 8
cores; per-core behaviour differs only through input data (rolled x, the
transposed residual, and a pad-row mask).

Per-core pipeline (all matmuls bf16 with fp32 PSUM accumulation):
  - q^T [64, 2176], k [64, 4096] projections.
  - v^T [4096, 513] (= x^T @ wv^T with a ones column appended).
  - energy computed TRANSPOSED: e^T[n,m] per 128-row n-chunk, exp on the
    Scalar engine straight from PSUM; the softmax denominator for each m
    falls out of the bmm against the ones column of v^T.
  - paT[m,c] accumulated over n-chunks, then normalized (per-partition
    reciprocal), residual-added and pad-masked in one pass; small PE
    transpose back to pa[c,m].
  - channel pooling partials + pair-wise AllReduce(add)/AllReduce(max),
    SE MLP + sigmoid on-chip (sigmoid via exp + reciprocal so only one
    activation table set is ever loaded).
  - 3x3 conv as 9 shifted-tap matmuls over a 66-column zero-padded layout,
    BN+ReLU fused into the final Scalar-engine activation.
"""

import numpy as np
import ml_dtypes

import concourse.bass as bass
import concourse.mybir as mybir
import concourse.tile as tile

BF16 = ml_dtypes.bfloat16
F32 = np.float32

P = 128
CIN = 512            # channels
NPIX = 4096          # 64*64 pixels
C8 = 64              # q/k channels
OC = 256             # conv output channels
M = 2176             # per-core pixel window: 34 rows * 64
NSUB = M // P        # 17
NCH = NPIX // P      # 32 n-chunks
BLOCKS = [(0, 512), (512, 512), (1024, 512), (1536, 512), (2048, 128)]
OWN_OFF = 64         # own pixels are window [64 : 64+2048] for every core
OWN = 2048
REPLICA_GROUPS = [[0, 1], [2, 3], [4, 5], [6, 7]]

BN_EPS = 1e-5

_BUILD_CACHE = {}


def _emit(tc, nc, t):
    """Emit the single-core program.  `t` maps input names -> dram handles."""
    fp32 = mybir.dt.float32
    bf16 = mybir.dt.bfloat16
    Act = mybir.ActivationFunctionType
    Alu = mybir.AluOpType
    XY = mybir.AxisListType.XY

    import contextlib
    ctx = contextlib.ExitStack()

    persist = ctx.enter_context(tc.tile_pool(name="persist", bufs=1))
    vt_pool = ctx.enter_context(tc.tile_pool(name="vt", bufs=NCH))
    xf_pool = ctx.enter_context(tc.tile_pool(name="xf", bufs=4))
    expt_pool = ctx.enter_context(tc.tile_pool(name="expt", bufs=3))
    patf_pool = ctx.enter_context(tc.tile_pool(name="patf", bufs=9))
    out_pool = ctx.enter_context(tc.tile_pool(name="yout", bufs=3))
    small = ctx.enter_context(tc.tile_pool(name="small", bufs=2))

    psum_e = ctx.enter_context(tc.tile_pool(name="ps_e", bufs=2, space="PSUM"))
    psum_pa = ctx.enter_context(tc.tile_pool(name="ps_pa", bufs=5, space="PSUM"))
    psum_d = ctx.enter_context(tc.tile_pool(name="ps_d", bufs=1, space="PSUM"))

    dram = ctx.enter_context(tc.tile_pool(name="dram", bufs=1, space="DRAM"))

    # ---------------- loads ----------------
    # xf: all first-halves before second-halves so q/k/vT start early
    xf_sb = []
    for ci in range(4):
        xt = xf_pool.tile([P, NPIX], bf16, tag="xf", name=f"xf{ci}")
        xf_sb.append(xt)
    for half in range(2):
        sl = slice(half * (NPIX // 2), (half + 1) * (NPIX // 2))
        for ci in range(4):
            nc.sync.dma_start(out=xf_sb[ci][:, sl],
                              in_=t["xf"][ci * P:(ci + 1) * P, sl])

    wqT_sb = persist.tile([P, 4, C8], bf16)
    nc.sync.dma_start(out=wqT_sb,
                      in_=t["wqT"].ap().rearrange("(c p) h -> p c h", p=P))
    wkT_sb = persist.tile([P, 4, C8], bf16)
    nc.sync.dma_start(out=wkT_sb,
                      in_=t["wkT"].ap().rearrange("(c p) h -> p c h", p=P))
    wvT_sb = persist.tile([P, 4, CIN], bf16)
    nc.sync.dma_start(out=wvT_sb,
                      in_=t["wvT"].ap().rearrange("(c p) n -> p c n", p=P))

    bq_sb = persist.tile([C8, 1], fp32)
    nc.sync.dma_start(out=bq_sb, in_=t["bq"][:, :])
    bk_sb = persist.tile([C8, 1], fp32)
    nc.sync.dma_start(out=bk_sb, in_=t["bk"][:, :])

    ident_sb = persist.tile([P, P], bf16)
    nc.sync.dma_start(out=ident_sb, in_=t["ident"][:, :])

    # ---------------- q / k projections ----------------
    # qT columns use the PERMUTED window order: own rows 1..32 first
    # (xf cols 64..2112), then halo rows 0 and 33 (cols 0..64, 2112..2176).
    qT_sb = persist.tile([C8, M], bf16)
    for off in range(0, OWN, 512):
        q_ps = psum_e.tile([C8, 512], fp32, tag="e")
        for ci in range(4):
            nc.tensor.matmul(q_ps, lhsT=wqT_sb[:, ci, :],
                             rhs=xf_sb[ci][:, 64 + off:64 + off + 512],
                             start=(ci == 0), stop=(ci == 3))
        nc.scalar.activation(qT_sb[:, off:off + 512], q_ps,
                             Act.Identity, bias=bq_sb[:, 0:1])
    qh_ps = psum_e.tile([C8, P], fp32, tag="e")
    for ci in range(4):
        nc.tensor.matmul(qh_ps[:, 0:64], lhsT=wqT_sb[:, ci, :],
                         rhs=xf_sb[ci][:, 0:64],
                         start=(ci == 0), stop=(ci == 3))
    for ci in range(4):
        nc.tensor.matmul(qh_ps[:, 64:128], lhsT=wqT_sb[:, ci, :],
                         rhs=xf_sb[ci][:, OWN + 64:OWN + 128],
                         start=(ci == 0), stop=(ci == 3))
    nc.scalar.activation(qT_sb[:, OWN:OWN + P], qh_ps,
                         Act.Identity, bias=bq_sb[:, 0:1])

    k_sb = persist.tile([C8, NPIX], bf16)
    for off in range(0, NPIX, 512):
        k_ps = psum_e.tile([C8, 512], fp32, tag="e")
        for ci in range(4):
            nc.tensor.matmul(k_ps, lhsT=wkT_sb[:, ci, :],
                             rhs=xf_sb[ci][:, off:off + 512],
                             start=(ci == 0), stop=(ci == 3))
        nc.scalar.activation(k_sb[:, off:off + 512], k_ps,
                             Act.Identity, bias=bk_sb[:, 0:1])

    # ---------------- v^T ----------------
    vt_sb = []
    for nch in range(NCH):
        v_ps = psum_e.tile([P, 512], fp32, tag="e")
        for ci in range(4):
            nc.tensor.matmul(v_ps,
                             lhsT=xf_sb[ci][:, nch * P:(nch + 1) * P],
                             rhs=wvT_sb[:, ci, :],
                             start=(ci == 0), stop=(ci == 3))
        vt = vt_pool.tile([P, CIN], bf16, tag="vt")
        nc.vector.tensor_copy(vt, v_ps)
        vt_sb.append(vt)

    # ---- late loads (not needed until block tails / conv) ----
    xtr_sb = persist.tile([P, NSUB, CIN], bf16)
    nc.sync.dma_start(
        out=xtr_sb,
        in_=t["xtr"].ap().rearrange("(mi p) c -> p mi c", p=P))

    pmask_sb = persist.tile([P, NSUB], fp32)
    nc.sync.dma_start(
        out=pmask_sb, in_=t["pmask"].ap().rearrange("(mi p) -> p mi", p=P))

    w1T_sb = persist.tile([P, 4, C8], bf16)
    nc.sync.dma_start(out=w1T_sb,
                      in_=t["w1T"].ap().rearrange("(c p) h -> p c h", p=P))
    w2T_sb = persist.tile([C8, 4, P], bf16)
    nc.sync.dma_start(out=w2T_sb,
                      in_=t["w2T"].ap().rearrange("k (c p) -> k c p", p=P))

    cw_sb = persist.tile([P, 36, OC], bf16)
    nc.sync.dma_start(out=cw_sb,
                      in_=t["cw"].ap().rearrange("t (c p) o -> p (t c) o", p=P))

    bns_sb = persist.tile([P, 2], fp32)
    nc.sync.dma_start(out=bns_sb,
                      in_=t["bns"].ap().rearrange("(c p) one -> p (c one)", p=P))
    bnb_sb = persist.tile([P, 2], fp32)
    nc.sync.dma_start(out=bnb_sb,
                      in_=t["bnb"].ap().rearrange("(c p) one -> p (c one)", p=P))

    # ---------------- position attention ----------------
    # ca: 34 rows x 66 cols, zero col pads.  Window-row r lives at row r.
    # Subchunk jg<16 covers rows (1+2jg, 2+2jg); subchunk 16 covers rows
    # 0 and 33 (the halo rows), so pooling (rows 1..32) completes before
    # the last attention block and the AllGather overlaps it.
    ca_sb = persist.tile([P, 4, 34 * 66], bf16)
    for cc in range(4):
        cav = ca_sb[:, cc, :].rearrange("p (r x) -> p r x", x=66)
        nc.vector.memset(cav[:, :, 0:1], 0.0)
        nc.vector.memset(cav[:, :, 65:66], 0.0)

    pool_s = small.tile([P, 4], fp32, tag="pool_s", bufs=1)
    pool_m = small.tile([P, 4], fp32, tag="pool_m", bufs=1)

    ones_f = small.tile([P, 1], fp32, tag="ones_f", bufs=1)
    nc.vector.memset(ones_f, 1.0)

    den_ps = psum_d.tile([P, 512], fp32, tag="den")
    zz = small.tile([P, 512], bf16, tag="zz", bufs=1)
    nc.vector.memset(zz, 0.0)
    nc.tensor.matmul(den_ps, lhsT=ident_sb, rhs=zz, start=True, stop=True)

    def pe_tail(bi, patfs):
        """PE transposes into ca + this block's pooling partials."""
        for jg, patf in patfs:
            tp_ps = psum_e.tile([P, 4, P], bf16, tag="e", name=f"tp{jg}")
            for cc in range(4):
                nc.tensor.transpose(tp_ps[:, cc, :],
                                    patf[:, cc * P:(cc + 1) * P], ident_sb)
            cav = ca_sb.rearrange("p c (r x) -> p c r x", x=66)
            tpv = tp_ps.rearrange("p c (r x) -> p c r x", x=64)
            if jg < 16:
                nc.vector.tensor_copy(
                    cav[:, :, 1 + 2 * jg:3 + 2 * jg, 1:65], tpv)
            else:
                nc.vector.tensor_copy(cav[:, :, 0:1, 1:65], tpv[:, :, 0:1, :])
                nc.vector.tensor_copy(cav[:, :, 33:34, 1:65],
                                      tpv[:, :, 1:2, :])
        if bi >= 4:
            return
        r0 = 1 + 8 * bi
        ptmp = small.tile([P, 4, 2], fp32, tag="ptmp")
        for cc in range(4):
            cav = ca_sb[:, cc, :].rearrange("p (r x) -> p r x", x=66)
            view = cav[:, r0:r0 + 8, 1:65]
            if bi == 0:
                nc.vector.reduce_sum(pool_s[:, cc:cc + 1], view, axis=XY)
                nc.vector.reduce_max(pool_m[:, cc:cc + 1], view, axis=XY)
            else:
                nc.vector.reduce_sum(ptmp[:, cc, 0:1], view, axis=XY)
                nc.vector.reduce_max(ptmp[:, cc, 1:2], view, axis=XY)
                nc.vector.tensor_add(pool_s[:, cc:cc + 1],
                                     pool_s[:, cc:cc + 1], ptmp[:, cc, 0:1])
                nc.vector.tensor_max(pool_m[:, cc:cc + 1],
                                     pool_m[:, cc:cc + 1], ptmp[:, cc, 1:2])

    def emit_loop(bi, boff, bsz, chunks):
        nsub = bsz // P
        if chunks.start == 0:
            st = [psum_pa.tile([P, CIN], fp32, tag="pa_acc", name=f"pa{j}")
                  for j in range(nsub)]
            es = patf_pool.tile([P, bsz], fp32, tag="esum", bufs=2)
            blk_state[bi] = (st, es)
        pa_ps, esum = blk_state[bi]
        for nch in chunks:
            e_ps = psum_e.tile([P, bsz], fp32, tag="e")
            nc.tensor.matmul(e_ps, lhsT=k_sb[:, nch * P:(nch + 1) * P],
                             rhs=qT_sb[:, boff:boff + bsz],
                             start=True, stop=True)
            expt = expt_pool.tile([P, bsz], bf16, tag="expt")
            nc.scalar.activation(expt, e_ps, Act.Exp)
            for j in range(nsub):
                nc.tensor.matmul(pa_ps[j][:, 0:CIN],
                                 lhsT=expt[:, j * P:(j + 1) * P],
                                 rhs=vt_sb[nch][:, 0:CIN],
                                 start=(nch == 0), stop=(nch == NCH - 1))
            if nch == 0:
                nc.vector.tensor_copy(esum, expt)
            else:
                nc.vector.tensor_add(esum, esum, expt)

    def dve_tail(bi, boff, bsz):
        """denominators (one matmul per subchunk), recip, normalize+residual."""
        nsub = bsz // P
        pa_ps, esum = blk_state[bi]
        for j in range(nsub):
            col = bi * 4 + j
            nc.tensor.matmul(den_ps[:, col:col + 1],
                             lhsT=esum[:, j * P:(j + 1) * P], rhs=ones_f,
                             start=False, stop=False, skip_group_check=True)
        recip = small.tile([P, 4], fp32, tag="recip")
        nc.vector.reciprocal(recip[:, 0:nsub],
                             den_ps[:, bi * 4:bi * 4 + nsub])
        jg0 = boff // P
        nc.vector.tensor_mul(recip[:, 0:nsub], recip[:, 0:nsub],
                             pmask_sb[:, jg0:jg0 + nsub])
        patfs = []
        for j in range(nsub):
            jg = jg0 + j
            pnorm = patf_pool.tile([P, CIN], fp32, tag="pnorm", bufs=3)
            nc.vector.tensor_scalar(out=pnorm, in0=pa_ps[j],
                                    scalar1=recip[:, j:j + 1], scalar2=None,
                                    op0=Alu.mult)
            patf = patf_pool.tile([P, CIN], bf16, tag="patf")
            nc.vector.tensor_add(patf, pnorm, xtr_sb[:, jg, :])
            patfs.append((jg, patf))
        return patfs

    blk_state = {}
    patfs_of = {}
    # blocks 0..3 (own rows), one-late PE tails
    for bi in range(4):
        boff, bsz = BLOCKS[bi]
        emit_loop(bi, boff, bsz, range(NCH))
        patfs_of[bi] = dve_tail(bi, boff, bsz)
        if bi >= 1:
            pe_tail(bi - 1, patfs_of.pop(bi - 1))
    # halo block: first half, then tail(3) + pooled-stats exchange, then rest
    boff, bsz = BLOCKS[4]
    emit_loop(4, boff, bsz, range(0, NCH // 2))
    pe_tail(3, patfs_of.pop(3))

    # ------- one pair AllGather of [sums | maxes], combined locally -------
    ag_in = dram.tile([2, CIN], fp32, tag="ag_in")
    ag_out = dram.tile([2, 2, CIN], fp32, tag="ag_out")
    pool_sm = small.tile([P, 2, 4], fp32, tag="pool_sm", bufs=1)
    nc.vector.tensor_copy(pool_sm[:, 0, :], pool_s)
    nc.vector.tensor_copy(pool_sm[:, 1, :], pool_m)
    nc.sync.dma_start(out=ag_in.rearrange("two (c p) -> p two c", p=P),
                      in_=pool_sm)
    nc.gpsimd.collective_compute("AllGather", Alu.bypass,
                                 replica_groups=REPLICA_GROUPS,
                                 ins=[ag_in.opt()], outs=[ag_out.opt()])

    emit_loop(4, boff, bsz, range(NCH // 2, NCH))
    patfs_of[4] = dve_tail(4, boff, bsz)
    pe_tail(4, patfs_of.pop(4))

    zall = small.tile([P, 2, 2, 4], fp32, tag="zall", bufs=1)
    nc.sync.dma_start(out=zall,
                      in_=ag_out.rearrange("m two (c p) -> p m two c", p=P))
    zs_sb = small.tile([P, 4], fp32, tag="zs")
    zm_sb = small.tile([P, 4], fp32, tag="zm")
    nc.vector.tensor_add(zs_sb, zall[:, 0, 0, :], zall[:, 1, 0, :])
    nc.vector.tensor_max(zm_sb, zall[:, 0, 1, :], zall[:, 1, 1, :])

    # ---------------- SE MLP + sigmoid ----------------
    rhs_z = small.tile([P, 4, 2], bf16, tag="rhs_z")
    nc.vector.tensor_scalar_mul(rhs_z[:, :, 0], zs_sb, 1.0 / float(NPIX))
    nc.vector.tensor_copy(rhs_z[:, :, 1], zm_sb)

    h_ps = psum_d.tile([C8, 2], fp32, tag="den")
    for cc in range(4):
        nc.tensor.matmul(h_ps, lhsT=w1T_sb[:, cc, :], rhs=rhs_z[:, cc, :],
                         start=(cc == 0), stop=(cc == 3))
    h_sb = small.tile([C8, 2], bf16, tag="h_sb")
    nc.vector.tensor_scalar_max(h_sb, h_ps, 0.0)

    stot = small.tile([P, 4], fp32, tag="stot")
    s_sb = small.tile([P, 4, 2], fp32, tag="s_sb", bufs=1)
    for cc in range(4):
        s_ps = psum_pa.tile([P, 2], fp32, tag="pa_acc")
        nc.tensor.matmul(s_ps, lhsT=w2T_sb[:, cc, :], rhs=h_sb,
                         start=True, stop=True)
        nc.vector.tensor_copy(s_sb[:, cc, :], s_ps)
        nc.vector.tensor_add(stot[:, cc:cc + 1], s_sb[:, cc, 0:1],
                             s_sb[:, cc, 1:2])

    es = small.tile([P, 4], fp32, tag="es")
    nc.scalar.activation(es, stot, Act.Exp, scale=-1.0)
    nc.vector.tensor_scalar_add(es, es, 1.0)
    scale_sb = small.tile([P, 4], fp32, tag="scale")
    nc.vector.reciprocal(scale_sb, es)

    # fold the per-input-channel SE scale into the conv weights
    cwS = persist.tile([P, 36, OC], bf16)
    cwv_in = cw_sb.rearrange("p (t c) o -> p c t o", c=4)
    cwv_out = cwS.rearrange("p (t c) o -> p c t o", c=4)
    for cc in range(4):
        nc.vector.tensor_scalar(out=cwv_out[:, cc], in0=cwv_in[:, cc],
                                scalar1=scale_sb[:, cc:cc + 1], scalar2=None,
                                op0=Alu.mult)

    # ---------------- conv 3x3 + BN + ReLU ----------------
    for pt in range(4):
        for oc in range(2):
            y_ps = psum_pa.tile([P, 512], fp32, tag="pa_acc")
            idx = 0
            for kh in range(3):
                for kw in range(3):
                    tnum = 3 * kh + kw
                    rs = 1 + 8 * pt + (kh - 1)
                    for ci in range(4):
                        rhs = (ca_sb[:, ci, :]
                               .rearrange("p (r x) -> p r x", x=66)
                               [:, rs:rs + 8, kw:kw + 64])
                        nc.tensor.matmul(
                            y_ps, lhsT=cwS[:, tnum * 4 + ci,
                                           oc * P:(oc + 1) * P],
                            rhs=rhs, start=(idx == 0), stop=(idx == 35))
                        idx += 1
            y_sb = out_pool.tile([P, 512], fp32, tag="y_sb")
            nc.scalar.activation(y_sb, y_ps, Act.Relu,
                                 bias=bnb_sb[:, oc:oc + 1],
                                 scale=bns_sb[:, oc:oc + 1])
            nc.sync.dma_start(
                out=t["out"][oc * P:(oc + 1) * P, pt * 512:(pt + 1) * 512],
                in_=y_sb)

    ctx.close()


def build():
    """Build (and cache) the SPMD Bass program."""
    if "nc" in _BUILD_CACHE:
        return _BUILD_CACHE["nc"]
    from concourse import bacc
    nc = bacc.Bacc("TRN2", target_bir_lowering=False, num_devices=8)
    f32 = mybir.dt.float32
    bf16 = mybir.dt.bfloat16
    t = {}
    t["xf"] = nc.dram_tensor("xf", [CIN, NPIX], bf16, kind="ExternalInput")
    t["xtr"] = nc.dram_tensor("xtr", [M, CIN], bf16, kind="ExternalInput")
    t["pmask"] = nc.dram_tensor("pmask", [M], f32, kind="ExternalInput")
    t["wqT"] = nc.dram_tensor("wqT", [CIN, C8], bf16, kind="ExternalInput")
    t["wkT"] = nc.dram_tensor("wkT", [CIN, C8], bf16, kind="ExternalInput")
    t["wvT"] = nc.dram_tensor("wvT", [CIN, CIN], bf16, kind="ExternalInput")
    t["bq"] = nc.dram_tensor("bq", [C8, 1], f32, kind="ExternalInput")
    t["bk"] = nc.dram_tensor("bk", [C8, 1], f32, kind="ExternalInput")
    t["w1T"] = nc.dram_tensor("w1T", [CIN, C8], bf16, kind="ExternalInput")
    t["w2T"] = nc.dram_tensor("w2T", [C8, CIN], bf16, kind="ExternalInput")
    t["cw"] = nc.dram_tensor("cw", [9, CIN, OC], bf16, kind="ExternalInput")
    t["bns"] = nc.dram_tensor("bns", [OC, 1], f32, kind="ExternalInput")
    t["bnb"] = nc.dram_tensor("bnb", [OC, 1], f32, kind="ExternalInput")
    t["ident"] = nc.dram_tensor("ident", [P, P], bf16, kind="ExternalInput")
    t["out"] = nc.dram_tensor("out", [OC, OWN], f32, kind="ExternalOutput")

    with tile.TileContext(nc) as tc:
        _emit(tc, nc, t)
    nc.compile()

    _BUILD_CACHE["nc"] = nc
    return nc


def make_in_maps(x, wq, bq, wk, bk, wv, bv, ca_w1, ca_w2, conv_w,
                 bn_gamma, bn_beta, bn_mean, bn_var):
    x = np.ascontiguousarray(np.asarray(x, F32))
    B = x.shape[0]
    xf_full = x.reshape(B, CIN, NPIX)

    common = {
        "wqT": np.ascontiguousarray(np.asarray(wq, F32).T.astype(BF16)),
        "wkT": np.ascontiguousarray(np.asarray(wk, F32).T.astype(BF16)),
        "wvT": np.ascontiguousarray(np.asarray(wv, F32).T.astype(BF16)),
        "bq": np.asarray(bq, F32).reshape(C8, 1),
        "bk": np.asarray(bk, F32).reshape(C8, 1),
        "w1T": np.ascontiguousarray(np.asarray(ca_w1, F32).T.astype(BF16)),
        "w2T": np.ascontiguousarray(np.asarray(ca_w2, F32).T.astype(BF16)),
        "cw": np.ascontiguousarray(np.stack(
            [np.asarray(conv_w, F32)[:, :, kh, kw].T
             for kh in range(3) for kw in range(3)]).astype(BF16)),
        "ident": np.eye(P, dtype=BF16),
    }
    bns = (np.asarray(bn_gamma, F32)
           / np.sqrt(np.asarray(bn_var, F32) + BN_EPS)).astype(F32)
    bnb = (np.asarray(bn_beta, F32) - np.asarray(bn_mean, F32) * bns).astype(F32)
    common["bns"] = bns.reshape(OC, 1)
    common["bnb"] = bnb.reshape(OC, 1)

    bv_f = np.asarray(bv, F32)
    in_maps = []
    for core in range(8):
        b, h = core // 2, core % 2
        r0 = 32 * h - 1                       # first window row (may be -1)
        rolled = np.roll(xf_full[b], -r0 * 64, axis=1)
        xtr = rolled[:, :M].T + bv_f[None, :]
        pmask = np.ones((M,), F32)
        if h == 0:
            xtr[0:64] = 0.0
            pmask[0:64] = 0.0
        else:
            xtr[M - 64:M] = 0.0
            pmask[M - 64:M] = 0.0
        # permuted window order: own rows 1..32 first, then halo rows 0, 33
        perm = np.concatenate([np.arange(64, OWN + 64),
                               np.arange(0, 64),
                               np.arange(OWN + 64, M)])
        xtr = xtr[perm]
        pmask = pmask[perm]
        in_maps.append(dict(
            common,
            xf=np.ascontiguousarray(rolled.astype(BF16)),
            xtr=np.ascontiguousarray(xtr.astype(BF16)),
            pmask=pmask,
        ))
    return in_maps


def assemble(results):
    out = np.zeros((4, OC, 64, 64), F32)
    for core in range(8):
        b, h = core // 2, core % 2
        out[b, :, 32 * h:32 * h + 32, :] = \
            results[core]["out"].reshape(OC, 32, 64)
    return out


def kernel(**inputs):
    from concourse.bass_utils import run_bass_kernel_spmd
    nc = build()
    in_maps = make_in_maps(**inputs)
    res = run_bass_kernel_spmd(nc, in_maps, core_ids=list(range(8)))
    return assemble(res.results)



# revision 11
# speedup vs baseline: 1.0775x; 1.0775x over previous
"""DANetHead (position attention + channel attention + conv/BN/ReLU) on 8
Trainium2 NeuronCores via Bass/Tile.

Sharding: data-parallel over batch (4) x image-row-halves (2) = 8 cores.
Each core computes a 34-row window (32 own rows + 1 halo row on each side)
of one batch item, position-uniform across cores via a host-side roll of
the pixel axis; per-core behaviour differs only through input data.

v2 restructure vs the first version:
  - pa is accumulated directly in [c, m] orientation (lhsT = vT c-chunk,
    rhs = expT) so the conv input layout falls out of the bmm with NO PE
    transposes and no separate transposed-residual input; the residual is
    the already-resident xf and the softmax normalization is applied as a
    per-column broadcast (ones-matmul) multiply.
  - q/k are projected with column-duplicated weights to [128, *] so the
    energy matmuls run as two concurrent 64-row tile_position matmuls
    (chunk pair per step) at ~2x effective rate.
  - channel pooling happens right in each block tail (free-axis reduce),
    so the pooled-stats AllGather is issued as soon as block 3 finishes;
    a dummy warmup AllGather at kernel start absorbs collective setup.
  - input DMAs split across the sync and scalar queues.
"""

import numpy as np
import ml_dtypes

import concourse.bass as bass
import concourse.mybir as mybir
import concourse.tile as tile

BF16 = ml_dtypes.bfloat16
F32 = np.float32

P = 128
CIN = 512            # channels
NPIX = 4096          # 64*64 pixels
C8 = 64              # q/k channels
OC = 256             # conv output channels
M = 2176             # per-core pixel window: 34 rows * 64
OWN = 2048           # own pixels: window rows 1..32 -> m 0..2047
NCH = NPIX // P      # 32 n-chunks
NPAIR = NCH // 2     # 16 chunk pairs
REPLICA_GROUPS = [[0, 1], [2, 3], [4, 5], [6, 7]]

BN_EPS = 1e-5

_BUILD_CACHE = {}


def _emit(tc, nc, t):
    fp32 = mybir.dt.float32
    f32r = mybir.dt.float32r
    bf16 = mybir.dt.bfloat16
    Act = mybir.ActivationFunctionType
    Alu = mybir.AluOpType
    AxX = mybir.AxisListType.X

    import contextlib
    ctx = contextlib.ExitStack()

    persist = ctx.enter_context(tc.tile_pool(name="persist", bufs=1))
    vt_pool = ctx.enter_context(tc.tile_pool(name="vt", bufs=NCH))
    expt_pool = ctx.enter_context(tc.tile_pool(name="expt", bufs=4))
    esum_pool = ctx.enter_context(tc.tile_pool(name="esum", bufs=2))
    t1_pool = ctx.enter_context(tc.tile_pool(name="t1", bufs=5))
    recb_pool = ctx.enter_context(tc.tile_pool(name="recb", bufs=2))
    out_pool = ctx.enter_context(tc.tile_pool(name="yout", bufs=3))
    small = ctx.enter_context(tc.tile_pool(name="small", bufs=2))

    ps_e = ctx.enter_context(tc.tile_pool(name="ps_e", bufs=2, space="PSUM"))
    ps_pa = ctx.enter_context(tc.tile_pool(name="ps_pa", bufs=5, space="PSUM"))
    ps_db = ctx.enter_context(tc.tile_pool(name="ps_db", bufs=1, space="PSUM"))

    dram = ctx.enter_context(tc.tile_pool(name="dram", bufs=1, space="DRAM"))

    # ---------------- warmup collective (absorb CC setup latency) -------
    wi_sb = small.tile([1, 8], fp32, tag="wi", bufs=1)
    nc.vector.memset(wi_sb, 0.0)
    warm_in = dram.tile([1, 8], fp32, tag="warm_in")
    warm_out = dram.tile([2, 8], fp32, tag="warm_out")
    nc.sync.dma_start(out=warm_in, in_=wi_sb)
    nc.gpsimd.collective_compute("AllGather", Alu.bypass,
                                 replica_groups=REPLICA_GROUPS,
                                 ins=[warm_in.opt()], outs=[warm_out.opt()])

    # ---------------- loads ----------------
    # xf on the sync queue; weights on the scalar queue (parallel DMA)
    xf_sb = [persist.tile([P, NPIX], bf16, name=f"xf{ci}") for ci in range(4)]
    for half in range(2):
        sl = slice(half * (NPIX // 2), (half + 1) * (NPIX // 2))
        for ci in range(4):
            nc.sync.dma_start(out=xf_sb[ci][:, sl],
                              in_=t["xf"][ci * P:(ci + 1) * P, sl])

    wq2_sb = persist.tile([P, 4, P], bf16)
    nc.scalar.dma_start(out=wq2_sb,
                        in_=t["wq2"].ap().rearrange("(c p) h -> p c h", p=P))
    wk2_sb = persist.tile([P, 4, P], bf16)
    nc.scalar.dma_start(out=wk2_sb,
                        in_=t["wk2"].ap().rearrange("(c p) h -> p c h", p=P))
    bq2_sb = persist.tile([P, 1], fp32)
    nc.scalar.dma_start(out=bq2_sb, in_=t["bq2"][:, :])
    bk2_sb = persist.tile([P, 1], fp32)
    nc.scalar.dma_start(out=bk2_sb, in_=t["bk2"][:, :])
    wvT_sb = persist.tile([P, 4, CIN], bf16)
    nc.scalar.dma_start(out=wvT_sb,
                        in_=t["wvT"].ap().rearrange("(c p) n -> p c n", p=P))

    # ---------------- q / k projections (row-duplicated to 128) --------
    # qT columns (m): own rows 1..32 -> xf cols 64..2112, then halo rows
    # 0, 33 -> xf cols 0..64 and 2112..2176.
    qT_sb = persist.tile([P, M], bf16)
    k_sb = persist.tile([P, NPIX], bf16)
    for off in range(0, 1536, 512):          # first-half-only q blocks
        q_ps = ps_e.tile([P, 512], fp32, tag="e")
        for ci in range(4):
            nc.tensor.matmul(q_ps, lhsT=wq2_sb[:, ci, :],
                             rhs=xf_sb[ci][:, 64 + off:64 + off + 512],
                             start=(ci == 0), stop=(ci == 3))
        nc.scalar.activation(qT_sb[:, off:off + 512], q_ps,
                             Act.Identity, bias=bq2_sb[:, 0:1])
    for off in range(0, 2048, 512):          # k first half
        k_ps = ps_e.tile([P, 512], fp32, tag="e")
        for ci in range(4):
            nc.tensor.matmul(k_ps, lhsT=wk2_sb[:, ci, :],
                             rhs=xf_sb[ci][:, off:off + 512],
                             start=(ci == 0), stop=(ci == 3))
        nc.scalar.activation(k_sb[:, off:off + 512], k_ps,
                             Act.Identity, bias=bk2_sb[:, 0:1])
    # q block 3 (needs xf cols 1600..2112 -> second half) + halo cols
    q_ps = ps_e.tile([P, 512], fp32, tag="e")
    for ci in range(4):
        nc.tensor.matmul(q_ps, lhsT=wq2_sb[:, ci, :],
                         rhs=xf_sb[ci][:, 64 + 1536:64 + 2048],
                         start=(ci == 0), stop=(ci == 3))
    nc.scalar.activation(qT_sb[:, 1536:2048], q_ps,
                         Act.Identity, bias=bq2_sb[:, 0:1])
    qh_ps = ps_e.tile([P, P], fp32, tag="e")
    for ci in range(4):
        nc.tensor.matmul(qh_ps[:, 0:64], lhsT=wq2_sb[:, ci, :],
                         rhs=xf_sb[ci][:, 0:64],
                         start=(ci == 0), stop=(ci == 3))
    for ci in range(4):
        nc.tensor.matmul(qh_ps[:, 64:128], lhsT=wq2_sb[:, ci, :],
                         rhs=xf_sb[ci][:, OWN + 64:OWN + 128],
                         start=(ci == 0), stop=(ci == 3))
    nc.scalar.activation(qT_sb[:, OWN:OWN + P], qh_ps,
                         Act.Identity, bias=bq2_sb[:, 0:1])
    for off in range(2048, NPIX, 512):       # k second half
        k_ps = ps_e.tile([P, 512], fp32, tag="e")
        for ci in range(4):
            nc.tensor.matmul(k_ps, lhsT=wk2_sb[:, ci, :],
                             rhs=xf_sb[ci][:, off:off + 512],
                             start=(ci == 0), stop=(ci == 3))
        nc.scalar.activation(k_sb[:, off:off + 512], k_ps,
                             Act.Identity, bias=bk2_sb[:, 0:1])

    # ---------------- v^T ----------------
    vt_sb = []
    for nch in range(NCH):
        v_ps = ps_e.tile([P, 512], fp32, tag="e")
        for ci in range(4):
            nc.tensor.matmul(v_ps,
                             lhsT=xf_sb[ci][:, nch * P:(nch + 1) * P],
                             rhs=wvT_sb[:, ci, :],
                             start=(ci == 0), stop=(ci == 3))
        vt = vt_pool.tile([P, CIN], bf16, tag="vt")
        nc.vector.tensor_copy(vt, v_ps)
        vt_sb.append(vt)

    # ---- late loads (scalar queue; not needed until tails / conv) ----
    xres_sb = persist.tile([P, 4, P], bf16)
    nc.scalar.dma_start(out=xres_sb,
                        in_=t["xres"].ap().rearrange("(c p) m -> p c m", p=P))
    pmask_sb = persist.tile([1, P], fp32)
    nc.scalar.dma_start(out=pmask_sb, in_=t["pmask"][:, :])
    bvp_sb = persist.tile([P, 4], fp32)
    nc.scalar.dma_start(out=bvp_sb,
                        in_=t["bvp"].ap().rearrange("(c p) one -> p (c one)",
                                                    p=P))
    w1T_sb = persist.tile([P, 4, C8], bf16)
    nc.scalar.dma_start(out=w1T_sb,
                        in_=t["w1T"].ap().rearrange("(c p) h -> p c h", p=P))
    w2T_sb = persist.tile([C8, 4, P], bf16)
    nc.scalar.dma_start(out=w2T_sb,
                        in_=t["w2T"].ap().rearrange("k (c p) -> k c p", p=P))
    cw_sb = persist.tile([P, 36, OC], bf16)
    nc.scalar.dma_start(out=cw_sb,
                        in_=t["cw"].ap().rearrange("t (c p) o -> p (t c) o",
                                                   p=P))
    bns_sb = persist.tile([P, 2], fp32)
    nc.scalar.dma_start(out=bns_sb,
                        in_=t["bns"].ap().rearrange("(c p) one -> p (c one)",
                                                    p=P))
    bnb_sb = persist.tile([P, 2], fp32)
    nc.scalar.dma_start(out=bnb_sb,
                        in_=t["bnb"].ap().rearrange("(c p) one -> p (c one)",
                                                    p=P))

    # ---------------- position attention ----------------
    # ca: [c-part, 4 c-groups, 34 rows x 66 cols], zero col pads.
    ca_sb = persist.tile([P, 4, 34 * 66], bf16)
    for cc in range(4):
        cav = ca_sb[:, cc, :].rearrange("p (r x) -> p r x", x=66)
        nc.vector.memset(cav[:, :, 0:1], 0.0)
        nc.vector.memset(cav[:, :, 65:66], 0.0)

    pool_s = small.tile([P, 4], fp32, tag="pool_s", bufs=1)
    pool_m = small.tile([P, 4], fp32, tag="pool_m", bufs=1)

    ones_col = small.tile([P, 1], fp32, tag="ones_c", bufs=1)
    nc.vector.memset(ones_col, 1.0)
    ones_row = small.tile([1, P], bf16, tag="ones_r", bufs=1)
    nc.vector.memset(ones_row, 1.0)

    def emit_block(bi, boff, bsz, halo):
        """One m-block: energy pairs + exp + pa accumulation + esum."""
        pa_ps = [ps_pa.tile([P, bsz], fp32, tag="pa_acc", name=f"pa{bi}_{cc}")
                 for cc in range(4)]
        esum = esum_pool.tile([P, bsz], fp32, tag="esum")
        prev = None
        for tp in range(NPAIR):
            e_a = ps_e.tile([P, bsz], fp32, tag="e")
            e_b = ps_e.tile([P, bsz], fp32, tag="e")
            n0, n1 = 2 * tp, 2 * tp + 1
            nc.tensor.matmul(e_a, lhsT=k_sb[0:64, n0 * P:(n0 + 1) * P],
                             rhs=qT_sb[0:64, boff:boff + bsz],
                             start=True, stop=True)
            nc.tensor.matmul(e_b, lhsT=k_sb[64:128, n1 * P:(n1 + 1) * P],
                             rhs=qT_sb[64:128, boff:boff + bsz],
                             start=True, stop=True)
            expt_a = expt_pool.tile([P, bsz], bf16, tag="expt")
            expt_b = expt_pool.tile([P, bsz], bf16, tag="expt")
            nc.scalar.activation(expt_a, e_a, Act.Exp)
            nc.scalar.activation(expt_b, e_b, Act.Exp)
            for cc in range(4):
                nc.tensor.matmul(pa_ps[cc],
                                 lhsT=vt_sb[n0][:, cc * P:(cc + 1) * P],
                                 rhs=expt_a, start=(tp == 0), stop=False)
            for cc in range(4):
                nc.tensor.matmul(pa_ps[cc],
                                 lhsT=vt_sb[n1][:, cc * P:(cc + 1) * P],
                                 rhs=expt_b, start=False, stop=(tp == NPAIR - 1))
            if tp == 0:
                nc.vector.tensor_copy(esum, expt_a)
            else:
                nc.vector.tensor_add(esum, esum, expt_a)
            nc.vector.tensor_add(esum, esum, expt_b)
        return pa_ps, esum

    def block_tail(bi, boff, bsz, pa_ps, esum, halo):
        """normalize (per-column), residual, pooling, ca write."""
        den_ps = ps_db.tile([1, bsz], fp32, tag="db")
        nc.tensor.matmul(den_ps, lhsT=ones_col, rhs=esum,
                         start=True, stop=True)
        recip_row = small.tile([1, bsz], bf16, tag="recip")
        with nc.allow_low_precision(reason="per-row softmax scale, bf16 ok"):
            nc.vector.reciprocal(recip_row, den_ps)
            if halo:
                nc.vector.tensor_mul(recip_row, recip_row, pmask_sb)
        recb_ps = ps_db.tile([P, bsz], fp32, tag="db")
        nc.tensor.matmul(recb_ps, lhsT=ones_row, rhs=recip_row,
                         start=True, stop=True)
        recb = recb_pool.tile([P, bsz], fp32, tag="recb")
        nc.vector.tensor_copy(recb, recb_ps)
        t1s = []
        for cc in range(4):
            t1 = t1_pool.tile([P, bsz], fp32, tag="t1")
            nc.vector.tensor_mul(t1, pa_ps[cc], recb)   # frees pa psum
            t1s.append(t1)
        cav = ca_sb.rearrange("p c (r x) -> p c r x", x=66)
        if not halo:
            ptmp = small.tile([P, 4, 2], fp32, tag="ptmp")
            for cc in range(4):
                t1 = t1s[cc]
                nc.vector.tensor_add(t1, t1,
                                     xf_sb[cc][:, 64 + boff:64 + boff + bsz])
                if bi == 0:
                    nc.vector.reduce_sum(pool_s[:, cc:cc + 1], t1, axis=AxX)
                    nc.vector.reduce_max(pool_m[:, cc:cc + 1], t1, axis=AxX)
                else:
                    nc.vector.reduce_sum(ptmp[:, cc, 0:1], t1, axis=AxX)
                    nc.vector.reduce_max(ptmp[:, cc, 1:2], t1, axis=AxX)
                    nc.vector.tensor_add(pool_s[:, cc:cc + 1],
                                         pool_s[:, cc:cc + 1], ptmp[:, cc, 0:1])
                    nc.vector.tensor_max(pool_m[:, cc:cc + 1],
                                         pool_m[:, cc:cc + 1], ptmp[:, cc, 1:2])
                r0 = 1 + 8 * bi
                nc.vector.tensor_scalar(
                    out=cav[:, cc, r0:r0 + 8, 1:65],
                    in0=t1.rearrange("p (r x) -> p r x", x=64),
                    scalar1=bvp_sb[:, cc:cc + 1], scalar2=None, op0=Alu.add)
        else:
            for cc in range(4):
                t1 = t1s[cc]
                nc.vector.tensor_add(t1, t1, xres_sb[:, cc, :])
                nc.vector.tensor_copy(cav[:, cc, 0:1, 1:65],
                                      t1[:, 0:64].rearrange("p (r x) -> p r x",
                                                            x=64))
                nc.vector.tensor_copy(cav[:, cc, 33:34, 1:65],
                                      t1[:, 64:128].rearrange("p (r x) -> p r x",
                                                              x=64))

    # blocks 0..3: own rows
    for bi in range(4):
        pa_ps, esum = emit_block(bi, 512 * bi, 512, halo=False)
        block_tail(bi, 512 * bi, 512, pa_ps, esum, halo=False)

    # ------- pooled-stats AllGather (issued right after block 3) -------
    ag_in = dram.tile([2, CIN], fp32, tag="ag_in")
    ag_out = dram.tile([2, 2, CIN], fp32, tag="ag_out")
    pool_sm = small.tile([P, 2, 4], fp32, tag="pool_sm", bufs=1)
    nc.vector.tensor_copy(pool_sm[:, 0, :], pool_s)
    nc.vector.tensor_copy(pool_sm[:, 1, :], pool_m)
    nc.sync.dma_start(out=ag_in.rearrange("two (c p) -> p two c", p=P),
                      in_=pool_sm)
    nc.gpsimd.collective_compute("AllGather", Alu.bypass,
                                 replica_groups=REPLICA_GROUPS,
                                 ins=[ag_in.opt()], outs=[ag_out.opt()])

    # halo block (rows 0 and 33) overlaps the collective
    pa_ps, esum = emit_block(4, OWN, P, halo=True)
    block_tail(4, OWN, P, pa_ps, esum, halo=True)

    zall = small.tile([P, 2, 2, 4], fp32, tag="zall", bufs=1)
    nc.sync.dma_start(out=zall,
                      in_=ag_out.rearrange("m two (c p) -> p m two c", p=P))
    zs_sb = small.tile([P, 4], fp32, tag="zs")
    zm_sb = small.tile([P, 4], fp32, tag="zm")
    nc.vector.tensor_add(zs_sb, zall[:, 0, 0, :], zall[:, 1, 0, :])
    nc.vector.tensor_max(zm_sb, zall[:, 0, 1, :], zall[:, 1, 1, :])

    # ---------------- SE MLP + sigmoid ----------------
    # pooled stats exclude bv (folded residual misses it): fix up here.
    rhs_z = small.tile([P, 4, 2], bf16, tag="rhs_z")
    zt = small.tile([P, 4], fp32, tag="zt")
    nc.vector.tensor_scalar_mul(zt, zs_sb, 1.0 / float(NPIX))
    nc.vector.tensor_add(rhs_z[:, :, 0], zt, bvp_sb)
    nc.vector.tensor_add(rhs_z[:, :, 1], zm_sb, bvp_sb)

    h_ps = ps_db.tile([C8, 2], fp32, tag="db")
    for cc in range(4):
        nc.tensor.matmul(h_ps, lhsT=w1T_sb[:, cc, :], rhs=rhs_z[:, cc, :],
                         start=(cc == 0), stop=(cc == 3))
    h_sb = small.tile([C8, 2], bf16, tag="h_sb")
    nc.vector.tensor_scalar_max(h_sb, h_ps, 0.0)

    stot = small.tile([P, 4], fp32, tag="stot")
    s_sb = small.tile([P, 4, 2], fp32, tag="s_sb", bufs=1)
    for cc in range(4):
        s_ps = ps_pa.tile([P, 2], fp32, tag="pa_acc")
        nc.tensor.matmul(s_ps, lhsT=w2T_sb[:, cc, :], rhs=h_sb,
                         start=True, stop=True)
        nc.vector.tensor_copy(s_sb[:, cc, :], s_ps)
        nc.vector.tensor_add(stot[:, cc:cc + 1], s_sb[:, cc, 0:1],
                             s_sb[:, cc, 1:2])

    es = small.tile([P, 4], fp32, tag="es")
    nc.scalar.activation(es, stot, Act.Exp, scale=-1.0)
    nc.vector.tensor_scalar_add(es, es, 1.0)
    scale_sb = small.tile([P, 4], fp32, tag="scale")
    nc.vector.reciprocal(scale_sb, es)

    # fold the per-input-channel SE scale into the conv weights
    # (split across the vector and scalar engines)
    cwS = persist.tile([P, 36, OC], bf16)
    cwv_in = cw_sb.rearrange("p (t c) o -> p c t o", c=4)
    cwv_out = cwS.rearrange("p (t c) o -> p c t o", c=4)
    for cc in range(4):
        if cc < 2:
            nc.vector.tensor_scalar(out=cwv_out[:, cc], in0=cwv_in[:, cc],
                                    scalar1=scale_sb[:, cc:cc + 1],
                                    scalar2=None, op0=Alu.mult)
        else:
            nc.scalar.activation(cwv_out[:, cc], cwv_in[:, cc],
                                 Act.Identity, scale=scale_sb[:, cc:cc + 1])

    # ---------------- conv 3x3 + BN + ReLU ----------------
    for pt in range(4):
        for oc in range(2):
            y_ps = ps_pa.tile([P, 512], fp32, tag="pa_acc")
            idx = 0
            for kh in range(3):
                for kw in range(3):
                    tnum = 3 * kh + kw
                    rs = 1 + 8 * pt + (kh - 1)
                    for ci in range(4):
                        rhs = (ca_sb[:, ci, :]
                               .rearrange("p (r x) -> p r x", x=66)
                               [:, rs:rs + 8, kw:kw + 64])
                        nc.tensor.matmul(
                            y_ps, lhsT=cwS[:, tnum * 4 + ci,
                                           oc * P:(oc + 1) * P],
                            rhs=rhs, start=(idx == 0), stop=(idx == 35))
                        idx += 1
            y_sb = out_pool.tile([P, 512], fp32, tag="y_sb")
            nc.scalar.activation(y_sb, y_ps, Act.Relu,
                                 bias=bnb_sb[:, oc:oc + 1],
                                 scale=bns_sb[:, oc:oc + 1])
            nc.sync.dma_start(
                out=t["out"][oc * P:(oc + 1) * P, pt * 512:(pt + 1) * 512],
                in_=y_sb)

    ctx.close()


def build():
    """Build (and cache) the SPMD Bass program."""
    if "nc" in _BUILD_CACHE:
        return _BUILD_CACHE["nc"]
    from concourse import bacc
    nc = bacc.Bacc("TRN2", target_bir_lowering=False, num_devices=8)
    f32 = mybir.dt.float32
    bf16 = mybir.dt.bfloat16
    t = {}
    t["xf"] = nc.dram_tensor("xf", [CIN, NPIX], bf16, kind="ExternalInput")
    t["xres"] = nc.dram_tensor("xres", [CIN, P], bf16, kind="ExternalInput")
    t["pmask"] = nc.dram_tensor("pmask", [1, P], f32, kind="ExternalInput")
    t["wq2"] = nc.dram_tensor("wq2", [CIN, P], bf16, kind="ExternalInput")
    t["wk2"] = nc.dram_tensor("wk2", [CIN, P], bf16, kind="ExternalInput")
    t["bq2"] = nc.dram_tensor("bq2", [P, 1], f32, kind="ExternalInput")
    t["bk2"] = nc.dram_tensor("bk2", [P, 1], f32, kind="ExternalInput")
    t["wvT"] = nc.dram_tensor("wvT", [CIN, CIN], bf16, kind="ExternalInput")
    t["bvp"] = nc.dram_tensor("bvp", [CIN, 1], f32, kind="ExternalInput")
    t["w1T"] = nc.dram_tensor("w1T", [CIN, C8], bf16, kind="ExternalInput")
    t["w2T"] = nc.dram_tensor("w2T", [C8, CIN], bf16, kind="ExternalInput")
    t["cw"] = nc.dram_tensor("cw", [9, CIN, OC], bf16, kind="ExternalInput")
    t["bns"] = nc.dram_tensor("bns", [OC, 1], f32, kind="ExternalInput")
    t["bnb"] = nc.dram_tensor("bnb", [OC, 1], f32, kind="ExternalInput")
    t["out"] = nc.dram_tensor("out", [OC, OWN], f32, kind="ExternalOutput")

    with tile.TileContext(nc) as tc:
        _emit(tc, nc, t)
    nc.compile()

    _BUILD_CACHE["nc"] = nc
    return nc


def make_in_maps(x, wq, bq, wk, bk, wv, bv, ca_w1, ca_w2, conv_w,
                 bn_gamma, bn_beta, bn_mean, bn_var):
    x = np.ascontiguousarray(np.asarray(x, F32))
    B = x.shape[0]
    xf_full = x.reshape(B, CIN, NPIX)

    wqT = np.asarray(wq, F32).T          # [CIN, C8]
    wkT = np.asarray(wk, F32).T
    common = {
        "wq2": np.ascontiguousarray(
            np.concatenate([wqT, wqT], axis=1).astype(BF16)),
        "wk2": np.ascontiguousarray(
            np.concatenate([wkT, wkT], axis=1).astype(BF16)),
        "bq2": np.tile(np.asarray(bq, F32).reshape(C8, 1), (2, 1)),
        "bk2": np.tile(np.asarray(bk, F32).reshape(C8, 1), (2, 1)),
        "wvT": np.ascontiguousarray(np.asarray(wv, F32).T.astype(BF16)),
        "bvp": np.asarray(bv, F32).reshape(CIN, 1),
        "w1T": np.ascontiguousarray(np.asarray(ca_w1, F32).T.astype(BF16)),
        "w2T": np.ascontiguousarray(np.asarray(ca_w2, F32).T.astype(BF16)),
        "cw": np.ascontiguousarray(np.stack(
            [np.asarray(conv_w, F32)[:, :, kh, kw].T
             for kh in range(3) for kw in range(3)]).astype(BF16)),
    }
    bns = (np.asarray(bn_gamma, F32)
           / np.sqrt(np.asarray(bn_var, F32) + BN_EPS)).astype(F32)
    bnb = (np.asarray(bn_beta, F32) - np.asarray(bn_mean, F32) * bns).astype(F32)
    common["bns"] = bns.reshape(OC, 1)
    common["bnb"] = bnb.reshape(OC, 1)

    bv_f = np.asarray(bv, F32)
    in_maps = []
    for core in range(8):
        b, h = core // 2, core % 2
        r0 = 32 * h - 1                       # first window row (may be -1)
        rolled = np.roll(xf_full[b], -r0 * 64, axis=1)
        # halo residual (+bv), zeroed on the pad row
        xres = np.empty((CIN, P), F32)
        xres[:, 0:64] = rolled[:, 0:64] + bv_f[:, None]      # window row 0
        xres[:, 64:128] = rolled[:, 2112:2176] + bv_f[:, None]  # row 33
        pmask = np.ones((1, P), F32)
        if h == 0:
            xres[:, 0:64] = 0.0
            pmask[0, 0:64] = 0.0
        else:
            xres[:, 64:128] = 0.0
            pmask[0, 64:128] = 0.0
        in_maps.append(dict(
            common,
            xf=np.ascontiguousarray(rolled.astype(BF16)),
            xres=np.ascontiguousarray(xres.astype(BF16)),
            pmask=pmask,
        ))
    return in_maps


def assemble(results):
    out = np.zeros((4, OC, 64, 64), F32)
    for core in range(8):
        b, h = core // 2, core % 2
        out[b, :, 32 * h:32 * h + 32, :] = \
            results[core]["out"].reshape(OC, 32, 64)
    return out


def kernel(**inputs):
    from concourse.bass_utils import run_bass_kernel_spmd
    nc = build()
    in_maps = make_in_maps(**inputs)
    res = run_bass_kernel_spmd(nc, in_maps, core_ids=list(range(8)))
    return assemble(res.results)


# revision 23
# speedup vs baseline: 1.1110x; 1.0311x over previous
"""DANetHead (position attention + channel attention + conv/BN/ReLU) on 8
Trainium2 NeuronCores via Bass/Tile.

Sharding: data-parallel over batch (4) x image-row-halves (2) = 8 cores.
Each core computes a 34-row window (32 own rows + 1 halo row on each side)
of one batch item, position-uniform across cores via a host-side roll of
the pixel axis; per-core behaviour differs only through input data.

v2 restructure vs the first version:
  - pa is accumulated directly in [c, m] orientation (lhsT = vT c-chunk,
    rhs = expT) so the conv input layout falls out of the bmm with NO PE
    transposes and no separate transposed-residual input; the residual is
    the already-resident xf and the softmax normalization is applied as a
    per-column broadcast (ones-matmul) multiply.
  - q/k are projected with column-duplicated weights to [128, *] so the
    energy matmuls run as two concurrent 64-row tile_position matmuls
    (chunk pair per step) at ~2x effective rate.
  - channel pooling happens right in each block tail (free-axis reduce),
    so the pooled-stats AllGather is issued as soon as block 3 finishes;
    a dummy warmup AllGather at kernel start absorbs collective setup.
  - input DMAs split across the sync and scalar queues.
"""

import numpy as np
import ml_dtypes

import concourse.bass as bass
import concourse.mybir as mybir
import concourse.tile as tile

BF16 = ml_dtypes.bfloat16
F32 = np.float32

P = 128
CIN = 512            # channels
NPIX = 4096          # 64*64 pixels
C8 = 64              # q/k channels
OC = 256             # conv output channels
M = 2176             # per-core pixel window: 34 rows * 64
OWN = 2048           # own pixels: window rows 1..32 -> m 0..2047
NCH = NPIX // P      # 32 n-chunks
NPAIR = NCH // 2     # 16 chunk pairs
REPLICA_GROUPS = [[0, 1], [2, 3], [4, 5], [6, 7]]

BN_EPS = 1e-5

_BUILD_CACHE = {}


def _emit(tc, nc, t):
    fp32 = mybir.dt.float32
    f32r = mybir.dt.float32r
    bf16 = mybir.dt.bfloat16
    Act = mybir.ActivationFunctionType
    Alu = mybir.AluOpType
    AxX = mybir.AxisListType.X

    import contextlib
    ctx = contextlib.ExitStack()

    persist = ctx.enter_context(tc.tile_pool(name="persist", bufs=1))
    vt_pool = ctx.enter_context(tc.tile_pool(name="vt", bufs=NCH))
    expt_pool = ctx.enter_context(tc.tile_pool(name="expt", bufs=4))
    esum_pool = ctx.enter_context(tc.tile_pool(name="esum", bufs=2))
    t1_pool = ctx.enter_context(tc.tile_pool(name="t1", bufs=5))
    recb_pool = ctx.enter_context(tc.tile_pool(name="recb", bufs=2))
    out_pool = ctx.enter_context(tc.tile_pool(name="yout", bufs=3))
    small = ctx.enter_context(tc.tile_pool(name="small", bufs=2))

    ps_e = ctx.enter_context(tc.tile_pool(name="ps_e", bufs=2, space="PSUM"))
    ps_pa = ctx.enter_context(tc.tile_pool(name="ps_pa", bufs=5, space="PSUM"))
    ps_db = ctx.enter_context(tc.tile_pool(name="ps_db", bufs=1, space="PSUM"))

    dram = ctx.enter_context(tc.tile_pool(name="dram", bufs=1, space="DRAM"))

    # ---------------- loads ----------------
    # xf on the sync queue (fine-grained so projections start early);
    # all weights on the gpsimd queue; NOTHING on the scalar queue so
    # the activation stream never stalls behind a DMA issue.
    wq2_sb = persist.tile([P, 4, P], bf16)
    nc.gpsimd.dma_start(out=wq2_sb,
                        in_=t["wq2"].ap().rearrange("(c p) h -> p c h", p=P))
    wk2_sb = persist.tile([P, 4, P], bf16)
    nc.gpsimd.dma_start(out=wk2_sb,
                        in_=t["wk2"].ap().rearrange("(c p) h -> p c h", p=P))
    bq2_sb = persist.tile([P, 1], fp32)
    nc.gpsimd.dma_start(out=bq2_sb, in_=t["bq2"][:, :])
    bk2_sb = persist.tile([P, 1], fp32)
    nc.gpsimd.dma_start(out=bk2_sb, in_=t["bk2"][:, :])

    xf_sb = [persist.tile([P, NPIX], bf16, name=f"xf{ci}") for ci in range(4)]
    for sl in (slice(0, 512), slice(512, 1024), slice(1024, 2048),
               slice(2048, 3072), slice(3072, 4096)):
        for ci in range(4):
            nc.sync.dma_start(out=xf_sb[ci][:, sl],
                              in_=t["xf"][ci * P:(ci + 1) * P, sl])

    wvT_sb = persist.tile([P, 4, CIN], bf16)
    nc.gpsimd.dma_start(out=wvT_sb,
                        in_=t["wvT"].ap().rearrange("(c p) n -> p c n", p=P))

    # warmup collective (absorb CC setup latency; result unused)
    wi_sb = small.tile([1, 8], fp32, tag="wi", bufs=1)
    nc.vector.memset(wi_sb, 0.0)
    warm_in = dram.tile([1, 8], fp32, tag="warm_in")
    warm_out = dram.tile([2, 8], fp32, tag="warm_out")
    nc.sync.dma_start(out=warm_in, in_=wi_sb)
    nc.gpsimd.collective_compute("AllGather", Alu.bypass,
                                 replica_groups=REPLICA_GROUPS,
                                 ins=[warm_in.opt()], outs=[warm_out.opt()])

    # ---------------- q / k projections (row-duplicated to 128) --------
    # qT columns (m): own rows 1..32 -> xf cols 64..2112, then halo rows
    # 0, 33 -> xf cols 0..64 and 2112..2176.
    qT_sb = persist.tile([P, M], bf16)
    k_sb = persist.tile([P, NPIX], bf16)
    for off in range(0, 1536, 512):          # first-half-only q blocks
        q_ps = ps_e.tile([P, 512], fp32, tag="e")
        for ci in range(4):
            nc.tensor.matmul(q_ps, lhsT=wq2_sb[:, ci, :],
                             rhs=xf_sb[ci][:, 64 + off:64 + off + 512],
                             start=(ci == 0), stop=(ci == 3))
        nc.scalar.activation(qT_sb[:, off:off + 512], q_ps,
                             Act.Identity, bias=bq2_sb[:, 0:1])
    for off in range(0, 2048, 512):          # k first half
        k_ps = ps_e.tile([P, 512], fp32, tag="e")
        for ci in range(4):
            nc.tensor.matmul(k_ps, lhsT=wk2_sb[:, ci, :],
                             rhs=xf_sb[ci][:, off:off + 512],
                             start=(ci == 0), stop=(ci == 3))
        nc.scalar.activation(k_sb[:, off:off + 512], k_ps,
                             Act.Identity, bias=bk2_sb[:, 0:1])
    # q block 3 (needs xf cols 1600..2112 -> second half) + halo cols
    q_ps = ps_e.tile([P, 512], fp32, tag="e")
    for ci in range(4):
        nc.tensor.matmul(q_ps, lhsT=wq2_sb[:, ci, :],
                         rhs=xf_sb[ci][:, 64 + 1536:64 + 2048],
                         start=(ci == 0), stop=(ci == 3))
    nc.scalar.activation(qT_sb[:, 1536:2048], q_ps,
                         Act.Identity, bias=bq2_sb[:, 0:1])
    qh_ps = ps_e.tile([P, P], fp32, tag="e")
    for ci in range(4):
        nc.tensor.matmul(qh_ps[:, 0:64], lhsT=wq2_sb[:, ci, :],
                         rhs=xf_sb[ci][:, 0:64],
                         start=(ci == 0), stop=(ci == 3))
    for ci in range(4):
        nc.tensor.matmul(qh_ps[:, 64:128], lhsT=wq2_sb[:, ci, :],
                         rhs=xf_sb[ci][:, OWN + 64:OWN + 128],
                         start=(ci == 0), stop=(ci == 3))
    nc.scalar.activation(qT_sb[:, OWN:OWN + P], qh_ps,
                         Act.Identity, bias=bq2_sb[:, 0:1])
    for off in range(2048, NPIX, 512):       # k second half
        k_ps = ps_e.tile([P, 512], fp32, tag="e")
        for ci in range(4):
            nc.tensor.matmul(k_ps, lhsT=wk2_sb[:, ci, :],
                             rhs=xf_sb[ci][:, off:off + 512],
                             start=(ci == 0), stop=(ci == 3))
        nc.scalar.activation(k_sb[:, off:off + 512], k_ps,
                             Act.Identity, bias=bk2_sb[:, 0:1])

    # ---------------- v^T ----------------
    vt_sb = []
    for nch in range(NCH):
        v_ps = ps_e.tile([P, 512], fp32, tag="e")
        for ci in range(4):
            nc.tensor.matmul(v_ps,
                             lhsT=xf_sb[ci][:, nch * P:(nch + 1) * P],
                             rhs=wvT_sb[:, ci, :],
                             start=(ci == 0), stop=(ci == 3))
        vt = vt_pool.tile([P, CIN], bf16, tag="vt")
        nc.vector.tensor_copy(vt, v_ps)
        vt_sb.append(vt)

    # ---- late loads (gpsimd queue; not needed until tails / conv) ----
    xres_sb = persist.tile([P, 4, P], bf16)
    nc.gpsimd.dma_start(out=xres_sb,
                        in_=t["xres"].ap().rearrange("(c p) m -> p c m", p=P))
    pmask_sb = persist.tile([1, P], bf16)
    nc.gpsimd.dma_start(out=pmask_sb, in_=t["pmask"][:, :])
    bvp_sb = persist.tile([P, 4], fp32)
    nc.gpsimd.dma_start(out=bvp_sb,
                        in_=t["bvp"].ap().rearrange("(c p) one -> p (c one)",
                                                    p=P))
    w1T_sb = persist.tile([P, 4, C8], bf16)
    nc.gpsimd.dma_start(out=w1T_sb,
                        in_=t["w1T"].ap().rearrange("(c p) h -> p c h", p=P))
    w2T_sb = persist.tile([C8, 4, P], bf16)
    nc.gpsimd.dma_start(out=w2T_sb,
                        in_=t["w2T"].ap().rearrange("k (c p) -> k c p", p=P))
    cw_sb = persist.tile([P, 36, OC], bf16)
    nc.gpsimd.dma_start(out=cw_sb,
                        in_=t["cw"].ap().rearrange("t (c p) o -> p (t c) o",
                                                   p=P))
    bns_sb = persist.tile([P, 2], fp32)
    nc.gpsimd.dma_start(out=bns_sb,
                        in_=t["bns"].ap().rearrange("(c p) one -> p (c one)",
                                                    p=P))
    bnb_sb = persist.tile([P, 2], fp32)
    nc.gpsimd.dma_start(out=bnb_sb,
                        in_=t["bnb"].ap().rearrange("(c p) one -> p (c one)",
                                                    p=P))

    # ---------------- position attention ----------------
    # ca: [c-part, 4 c-groups, 34 rows x 66 cols], zero col pads.
    ca_sb = persist.tile([P, 4, 34 * 66], bf16)
    for cc in range(4):
        cav = ca_sb[:, cc, :].rearrange("p (r x) -> p r x", x=66)
        nc.vector.memset(cav[:, :, 0:1], 0.0)
        nc.vector.memset(cav[:, :, 65:66], 0.0)

    pool_s = small.tile([P, 4], fp32, tag="pool_s", bufs=1)
    pool_m = small.tile([P, 4], fp32, tag="pool_m", bufs=1)
    pool_sm = small.tile([P, P], bf16, tag="pool_sm", bufs=1)
    nc.vector.memset(pool_sm, 0.0)

    ones_col = small.tile([P, 1], fp32, tag="ones_c", bufs=1)
    nc.vector.memset(ones_col, 1.0)
    ones_row = small.tile([1, P], bf16, tag="ones_r", bufs=1)
    nc.vector.memset(ones_row, 1.0)

    # pad-column mask broadcast to [P, P] once (off the critical path)
    pmb_ps = ps_db.tile([P, P], fp32, tag="db")
    nc.tensor.matmul(pmb_ps, lhsT=ones_row, rhs=pmask_sb,
                     start=True, stop=True)
    pmask_bc = small.tile([P, P], bf16, tag="pmask_bc", bufs=1)
    nc.vector.tensor_copy(pmask_bc, pmb_ps)

    def emit_block(bi, boff, bsz, halo):
        """One m-block: energy pairs + exp + pa accumulation + esum."""
        pa_ps = [ps_pa.tile([P, bsz], fp32, tag="pa_acc", name=f"pa{bi}_{cc}")
                 for cc in range(4)]
        esum = esum_pool.tile([P, bsz], fp32, tag="esum")
        prev = None
        for tp in range(NPAIR):
            e_a = ps_e.tile([P, bsz], fp32, tag="e")
            e_b = ps_e.tile([P, bsz], fp32, tag="e")
            n0, n1 = 2 * tp, 2 * tp + 1
            nc.tensor.matmul(e_a, lhsT=k_sb[0:64, n0 * P:(n0 + 1) * P],
                             rhs=qT_sb[0:64, boff:boff + bsz],
                             start=True, stop=True)
            nc.tensor.matmul(e_b, lhsT=k_sb[64:128, n1 * P:(n1 + 1) * P],
                             rhs=qT_sb[64:128, boff:boff + bsz],
                             start=True, stop=True)
            expt_a = expt_pool.tile([P, bsz], bf16, tag="expt")
            expt_b = expt_pool.tile([P, bsz], bf16, tag="expt")
            nc.scalar.activation(expt_a, e_a, Act.Exp)
            nc.scalar.activation(expt_b, e_b, Act.Exp)
            for cc in range(4):
                nc.tensor.matmul(pa_ps[cc],
                                 lhsT=vt_sb[n0][:, cc * P:(cc + 1) * P],
                                 rhs=expt_a, start=(tp == 0), stop=False)
            for cc in range(4):
                nc.tensor.matmul(pa_ps[cc],
                                 lhsT=vt_sb[n1][:, cc * P:(cc + 1) * P],
                                 rhs=expt_b, start=False, stop=(tp == NPAIR - 1))
            if tp == 0:
                nc.vector.tensor_copy(esum, expt_a)
            else:
                nc.vector.tensor_add(esum, esum, expt_a)
            nc.vector.tensor_add(esum, esum, expt_b)
        return pa_ps, esum

    def block_tail(bi, boff, bsz, pa_ps, esum, halo):
        """normalize (per-column), residual, pooling, ca write."""
        den_ps = ps_db.tile([1, bsz], fp32, tag="db")
        nc.tensor.matmul(den_ps, lhsT=ones_col, rhs=esum,
                         start=True, stop=True)
        den_row = small.tile([1, bsz], bf16, tag="den_row")
        nc.vector.tensor_copy(den_row, den_ps)
        recb_ps = ps_db.tile([P, bsz], fp32, tag="db")
        nc.tensor.matmul(recb_ps, lhsT=ones_row, rhs=den_row,
                         start=True, stop=True)
        recb = recb_pool.tile([P, bsz], fp32, tag="recb")
        nc.vector.reciprocal(recb, recb_ps)
        if halo:
            nc.vector.tensor_mul(recb, recb, pmask_bc)
        t1s = []
        for cc in range(4):
            t1 = t1_pool.tile([P, bsz], fp32, tag="t1")
            nc.vector.tensor_mul(t1, pa_ps[cc], recb)   # frees pa psum
            t1s.append(t1)
        cav = ca_sb.rearrange("p c (r x) -> p c r x", x=66)
        if not halo:
            ptmp = small.tile([P, 4, 2], fp32, tag="ptmp")
            for cc in range(4):
                t1 = t1s[cc]
                nc.vector.tensor_add(t1, t1,
                                     xf_sb[cc][:, 64 + boff:64 + boff + bsz])
                if bi == 0:
                    nc.vector.reduce_sum(pool_s[:, cc:cc + 1], t1, axis=AxX)
                    nc.vector.reduce_max(pool_m[:, cc:cc + 1], t1, axis=AxX)
                else:
                    nc.vector.reduce_sum(ptmp[:, cc, 0:1], t1, axis=AxX)
                    nc.vector.reduce_max(ptmp[:, cc, 1:2], t1, axis=AxX)
                    nc.vector.tensor_add(pool_s[:, cc:cc + 1],
                                         pool_s[:, cc:cc + 1], ptmp[:, cc, 0:1])
                    nc.vector.tensor_max(pool_m[:, cc:cc + 1],
                                         pool_m[:, cc:cc + 1], ptmp[:, cc, 1:2])
                r0 = 1 + 8 * bi
                nc.vector.tensor_scalar(
                    out=cav[:, cc, r0:r0 + 8, 1:65],
                    in0=t1.rearrange("p (r x) -> p r x", x=64),
                    scalar1=bvp_sb[:, cc:cc + 1], scalar2=None, op0=Alu.add)
        else:
            for cc in range(4):
                t1 = t1s[cc]
                nc.vector.tensor_add(t1, t1, xres_sb[:, cc, :])
                nc.vector.tensor_copy(cav[:, cc, 0:1, 1:65],
                                      t1[:, 0:64].rearrange("p (r x) -> p r x",
                                                            x=64))
                nc.vector.tensor_copy(cav[:, cc, 33:34, 1:65],
                                      t1[:, 64:128].rearrange("p (r x) -> p r x",
                                                              x=64))

    # blocks 0..3: own rows
    for bi in range(4):
        pa_ps, esum = emit_block(bi, 512 * bi, 512, halo=False)
        block_tail(bi, 512 * bi, 512, pa_ps, esum, halo=False)

    # ------- pooled-stats AllGather (issued right after block 3) -------
    # Transposed to [8, P] rows so the DRAM DMAs are contiguous packets
    # instead of a 128-partition scatter.
    ag_in = dram.tile([8, P], bf16, tag="ag_in")
    ag_out = dram.tile([2, 8, P], bf16, tag="ag_out")
    nc.vector.tensor_copy(pool_sm[:, 0:4], pool_s)
    nc.vector.tensor_copy(pool_sm[:, 4:8], pool_m)
    pool_smT = small.tile([P, P], bf16, tag="pool_smT", bufs=1)
    nc.sync.dma_start(out=pool_smT, in_=pool_sm, transpose=True)
    nc.sync.dma_start(out=ag_in, in_=pool_smT[0:8, :])
    nc.gpsimd.collective_compute("AllGather", Alu.bypass,
                                 replica_groups=REPLICA_GROUPS,
                                 ins=[ag_in.opt()], outs=[ag_out.opt()])

    # halo block (rows 0 and 33) overlaps the collective
    pa_ps, esum = emit_block(4, OWN, P, halo=True)
    block_tail(4, OWN, P, pa_ps, esum, halo=True)

    agT = small.tile([16, P], bf16, tag="agT", bufs=1)
    nc.sync.dma_start(out=agT, in_=ag_out.rearrange("m e p -> (m e) p"))
    zall = small.tile([P, 16], bf16, tag="zall", bufs=1)
    nc.sync.dma_start(out=zall, in_=agT, transpose=True)
    zv = zall.rearrange("p (m s c) -> p m s c", m=2, s=2)
    zs_sb = small.tile([P, 4], fp32, tag="zs")
    zm_sb = small.tile([P, 4], fp32, tag="zm")
    nc.vector.tensor_add(zs_sb, zv[:, 0, 0, :], zv[:, 1, 0, :])
    nc.vector.tensor_max(zm_sb, zv[:, 0, 1, :], zv[:, 1, 1, :])

    # ---------------- SE MLP + sigmoid ----------------
    # pooled stats exclude bv (folded residual misses it): fix up here.
    rhs_z = small.tile([P, 4, 2], bf16, tag="rhs_z")
    zt = small.tile([P, 4], fp32, tag="zt")
    nc.vector.tensor_scalar_mul(zt, zs_sb, 1.0 / float(NPIX))
    nc.vector.tensor_add(rhs_z[:, :, 0], zt, bvp_sb)
    nc.vector.tensor_add(rhs_z[:, :, 1], zm_sb, bvp_sb)

    h_ps = ps_db.tile([C8, 2], fp32, tag="db")
    for cc in range(4):
        nc.tensor.matmul(h_ps, lhsT=w1T_sb[:, cc, :], rhs=rhs_z[:, cc, :],
                         start=(cc == 0), stop=(cc == 3))
    h_sb = small.tile([C8, 2], bf16, tag="h_sb")
    nc.vector.tensor_scalar_max(h_sb, h_ps, 0.0)

    stot = small.tile([P, 4], fp32, tag="stot")
    s_sb = small.tile([P, 4, 2], fp32, tag="s_sb", bufs=1)
    for cc in range(4):
        s_ps = ps_pa.tile([P, 2], fp32, tag="pa_acc")
        nc.tensor.matmul(s_ps, lhsT=w2T_sb[:, cc, :], rhs=h_sb,
                         start=True, stop=True)
        nc.vector.tensor_copy(s_sb[:, cc, :], s_ps)
        nc.vector.tensor_add(stot[:, cc:cc + 1], s_sb[:, cc, 0:1],
                             s_sb[:, cc, 1:2])

    es = small.tile([P, 4], fp32, tag="es")
    nc.scalar.activation(es, stot, Act.Exp, scale=-1.0)
    nc.vector.tensor_scalar_add(es, es, 1.0)
    scale_sb = small.tile([P, 4], fp32, tag="scale")
    nc.vector.reciprocal(scale_sb, es)

    # fold the per-input-channel SE scale into the conv weights
    # (split across the vector and scalar engines)
    cwS = persist.tile([P, 36, OC], bf16)
    cwv_in = cw_sb.rearrange("p (t c) o -> p c t o", c=4)
    cwv_out = cwS.rearrange("p (t c) o -> p c t o", c=4)
    for cc in range(4):
        if cc < 2:
            nc.vector.tensor_scalar(out=cwv_out[:, cc], in0=cwv_in[:, cc],
                                    scalar1=scale_sb[:, cc:cc + 1],
                                    scalar2=None, op0=Alu.mult)
        else:
            nc.scalar.activation(cwv_out[:, cc], cwv_in[:, cc],
                                 Act.Identity, scale=scale_sb[:, cc:cc + 1])

    # ---------------- conv 3x3 + BN + ReLU ----------------
    for pt in range(4):
        for oc in range(2):
            y_ps = ps_pa.tile([P, 512], fp32, tag="pa_acc")
            idx = 0
            for kh in range(3):
                for kw in range(3):
                    tnum = 3 * kh + kw
                    rs = 1 + 8 * pt + (kh - 1)
                    for ci in range(4):
                        rhs = (ca_sb[:, ci, :]
                               .rearrange("p (r x) -> p r x", x=66)
                               [:, rs:rs + 8, kw:kw + 64])
                        nc.tensor.matmul(
                            y_ps, lhsT=cwS[:, tnum * 4 + ci,
                                           oc * P:(oc + 1) * P],
                            rhs=rhs, start=(idx == 0), stop=(idx == 35))
                        idx += 1
            y_sb = out_pool.tile([P, 512], fp32, tag="y_sb")
            nc.scalar.activation(y_sb, y_ps, Act.Relu,
                                 bias=bnb_sb[:, oc:oc + 1],
                                 scale=bns_sb[:, oc:oc + 1])
            nc.sync.dma_start(
                out=t["out"][oc * P:(oc + 1) * P, pt * 512:(pt + 1) * 512],
                in_=y_sb)

    ctx.close()


def build():
    """Build (and cache) the SPMD Bass program."""
    if "nc" in _BUILD_CACHE:
        return _BUILD_CACHE["nc"]
    from concourse import bacc
    nc = bacc.Bacc("TRN2", target_bir_lowering=False, num_devices=8)
    f32 = mybir.dt.float32
    bf16 = mybir.dt.bfloat16
    t = {}
    t["xf"] = nc.dram_tensor("xf", [CIN, NPIX], bf16, kind="ExternalInput")
    t["xres"] = nc.dram_tensor("xres", [CIN, P], bf16, kind="ExternalInput")
    t["pmask"] = nc.dram_tensor("pmask", [1, P], bf16, kind="ExternalInput")
    t["wq2"] = nc.dram_tensor("wq2", [CIN, P], bf16, kind="ExternalInput")
    t["wk2"] = nc.dram_tensor("wk2", [CIN, P], bf16, kind="ExternalInput")
    t["bq2"] = nc.dram_tensor("bq2", [P, 1], f32, kind="ExternalInput")
    t["bk2"] = nc.dram_tensor("bk2", [P, 1], f32, kind="ExternalInput")
    t["wvT"] = nc.dram_tensor("wvT", [CIN, CIN], bf16, kind="ExternalInput")
    t["bvp"] = nc.dram_tensor("bvp", [CIN, 1], f32, kind="ExternalInput")
    t["w1T"] = nc.dram_tensor("w1T", [CIN, C8], bf16, kind="ExternalInput")
    t["w2T"] = nc.dram_tensor("w2T", [C8, CIN], bf16, kind="ExternalInput")
    t["cw"] = nc.dram_tensor("cw", [9, CIN, OC], bf16, kind="ExternalInput")
    t["bns"] = nc.dram_tensor("bns", [OC, 1], f32, kind="ExternalInput")
    t["bnb"] = nc.dram_tensor("bnb", [OC, 1], f32, kind="ExternalInput")
    t["out"] = nc.dram_tensor("out", [OC, OWN], f32, kind="ExternalOutput")

    with tile.TileContext(nc) as tc:
        _emit(tc, nc, t)
    nc.compile()

    _BUILD_CACHE["nc"] = nc
    return nc


def make_in_maps(x, wq, bq, wk, bk, wv, bv, ca_w1, ca_w2, conv_w,
                 bn_gamma, bn_beta, bn_mean, bn_var):
    x = np.ascontiguousarray(np.asarray(x, F32))
    B = x.shape[0]
    xf_full = x.reshape(B, CIN, NPIX)

    wqT = np.asarray(wq, F32).T          # [CIN, C8]
    wkT = np.asarray(wk, F32).T
    common = {
        "wq2": np.ascontiguousarray(
            np.concatenate([wqT, wqT], axis=1).astype(BF16)),
        "wk2": np.ascontiguousarray(
            np.concatenate([wkT, wkT], axis=1).astype(BF16)),
        "bq2": np.tile(np.asarray(bq, F32).reshape(C8, 1), (2, 1)),
        "bk2": np.tile(np.asarray(bk, F32).reshape(C8, 1), (2, 1)),
        "wvT": np.ascontiguousarray(np.asarray(wv, F32).T.astype(BF16)),
        "bvp": np.asarray(bv, F32).reshape(CIN, 1),
        "w1T": np.ascontiguousarray(np.asarray(ca_w1, F32).T.astype(BF16)),
        "w2T": np.ascontiguousarray(np.asarray(ca_w2, F32).T.astype(BF16)),
        "cw": np.ascontiguousarray(np.stack(
            [np.asarray(conv_w, F32)[:, :, kh, kw].T
             for kh in range(3) for kw in range(3)]).astype(BF16)),
    }
    bns = (np.asarray(bn_gamma, F32)
           / np.sqrt(np.asarray(bn_var, F32) + BN_EPS)).astype(F32)
    bnb = (np.asarray(bn_beta, F32) - np.asarray(bn_mean, F32) * bns).astype(F32)
    common["bns"] = bns.reshape(OC, 1)
    common["bnb"] = bnb.reshape(OC, 1)

    bv_f = np.asarray(bv, F32)
    in_maps = []
    for core in range(8):
        b, h = core // 2, core % 2
        r0 = 32 * h - 1                       # first window row (may be -1)
        rolled = np.roll(xf_full[b], -r0 * 64, axis=1)
        # halo residual (+bv), zeroed on the pad row
        xres = np.empty((CIN, P), F32)
        xres[:, 0:64] = rolled[:, 0:64] + bv_f[:, None]      # window row 0
        xres[:, 64:128] = rolled[:, 2112:2176] + bv_f[:, None]  # row 33
        pmask = np.ones((1, P), F32)
        if h == 0:
            xres[:, 0:64] = 0.0
            pmask[0, 0:64] = 0.0
        else:
            xres[:, 64:128] = 0.0
            pmask[0, 64:128] = 0.0
        in_maps.append(dict(
            common,
            xf=np.ascontiguousarray(rolled.astype(BF16)),
            xres=np.ascontiguousarray(xres.astype(BF16)),
            pmask=pmask.astype(BF16),
        ))
    return in_maps


def assemble(results):
    out = np.zeros((4, OC, 64, 64), F32)
    for core in range(8):
        b, h = core // 2, core % 2
        out[b, :, 32 * h:32 * h + 32, :] = \
            results[core]["out"].reshape(OC, 32, 64)
    return out


def kernel(**inputs):
    from concourse.bass_utils import run_bass_kernel_spmd
    nc = build()
    in_maps = make_in_maps(**inputs)
    res = run_bass_kernel_spmd(nc, in_maps, core_ids=list(range(8)))
    return assemble(res.results)
